# revision 21
# baseline (speedup 1.0000x reference)
import os

os.environ.setdefault("JAX_PLATFORMS", "cpu,axon")
os.environ.setdefault("JAX_COMPILATION_CACHE_DIR", "/tmp/jax_pcache")
os.environ.setdefault("JAX_PERSISTENT_CACHE_MIN_COMPILE_TIME_SECS", "1")
import numpy as np

DEVICE_OK = False
LAST_EXEC_NS = None

HEADS = 8
DH_QK = 32
DH_V = 32
BS = 8
HALO = 3
WIN = BS + 2 * HALO   # 14
REL = 2 * WIN - 1     # 27
SCALE = DH_QK ** -0.5

# Per-core shard: core c handles image c//2, row-half c%2 (64 rows x 128 cols).
# The device kernel computes the fused projection
# out[OC, 8192] = W[OC,256] @ x[256, 8192] for that half image, in bf16.
# OC = 768 (q,k,v) + 448 (per-head rel-H 27 + rel-W 27, padded to 56) = 1216.
ROWS = 64
POS = ROWS * 128          # 8192
NT = POS // 512           # 16 N-tiles
OC_REL = 512              # 8 heads x 64 (27 H + 1 pad + 27 W + 9 pad)
OC = 768 + OC_REL         # 1280 output channels
assert OC % 128 == 0


def _build_nc():
    import concourse.mybir as mybir
    import concourse.tile as tile
    from concourse import bacc

    bf16 = mybir.dt.bfloat16
    nc = bacc.Bacc("TRN2", target_bir_lowering=False)
    x = nc.dram_tensor("x", [128, 2, POS], bf16, kind="ExternalInput")
    wt = nc.dram_tensor("wt", [128, 2, OC], bf16, kind="ExternalInput")
    out = nc.dram_tensor("qkv", [OC, POS], bf16, kind="ExternalOutput")

    n_oct = OC // 128
    with tile.TileContext(nc) as tc:
        with (
            tc.tile_pool(name="wp", bufs=1) as wp,
            tc.tile_pool(name="sb", bufs=3) as sb,
            tc.tile_pool(name="ob", bufs=1) as ob,
            tc.tile_pool(name="pp", bufs=8, space="PSUM") as pp,
        ):
            # one DMA for all weights: [128, 2*OC]
            wtile = wp.tile([128, 2 * OC], bf16, tag="w")
            nc.sync.dma_start(wtile[:, :], wt[:, :, :])
            # one big staging buffer: no tile reuse -> copies never wait on
            # an out-DMA (keeps every compute inst at <=2 sem waits, the
            # walrus per-instruction limit).
            ot_all = ob.tile([128, NT * n_oct * 512], bf16, tag="o")
            for nt in range(NT):
                # one DMA for both K-chunks of x: [128, 2*512]
                xt = sb.tile([128, 2 * 512], bf16, tag="x")
                nc.sync.dma_start(
                    xt[:, :], x[:, :, nt * 512:(nt + 1) * 512]
                )
                for oc in range(n_oct):
                    ps = pp.tile([128, 512], mybir.dt.float32, tag="ps")
                    for kc in range(2):
                        nc.tensor.matmul(
                            ps[:, :],
                            wtile[:, kc * OC + oc * 128:kc * OC + (oc + 1) * 128],
                            xt[:, kc * 512:(kc + 1) * 512],
                            start=(kc == 0),
                            stop=(kc == 1),
                        )
                    seg = (nt * n_oct + oc) * 512
                    nc.scalar.copy(ot_all[:, seg:seg + 512], ps[:, :])
                    nc.sync.dma_start(
                        out[oc * 128:(oc + 1) * 128, nt * 512:(nt + 1) * 512],
                        ot_all[:, seg:seg + 512],
                    )
    nc.compile()
    return nc


_NC_CACHE = None


def _project_on_device(x, w_full):
    """x: (4,256,128,128) f32, w_full: (OC,256) f32 -> (4,OC,128,128) f32."""
    import ml_dtypes
    from concourse.bass_utils import run_bass_kernel_spmd

    global _NC_CACHE, LAST_EXEC_NS
    if _NC_CACHE is None:
        _NC_CACHE = _build_nc()
    nc = _NC_CACHE
    bf = ml_dtypes.bfloat16
    wt = np.ascontiguousarray(
        w_full.T.reshape(2, 128, OC).transpose(1, 0, 2)
    ).astype(bf)
    in_maps = []
    for c in range(8):
        b, h = c // 2, c % 2
        xs = np.ascontiguousarray(
            x[b, :, h * ROWS:(h + 1) * ROWS, :]
            .reshape(2, 128, POS)
            .transpose(1, 0, 2)
        ).astype(bf)
        in_maps.append({"x": xs, "wt": wt})
    res = run_bass_kernel_spmd(nc, in_maps, core_ids=list(range(8)))
    if getattr(res, "exec_time_ns", None):
        LAST_EXEC_NS = res.exec_time_ns
    qkv = np.empty((4, OC, 128, 128), np.float32)
    for c in range(8):
        b, h = c // 2, c % 2
        qkv[b, :, h * ROWS:(h + 1) * ROWS, :] = res.results[c]["qkv"].astype(
            np.float32
        ).reshape(OC, ROWS, 128)
    return qkv


def kernel(x, w_q, w_kv, height_rel, width_rel):
    global DEVICE_OK
    x = np.asarray(x, np.float32)
    w_q = np.asarray(w_q, np.float32)
    w_kv = np.asarray(w_kv, np.float32)
    height_rel = np.asarray(height_rel, np.float32)
    width_rel = np.asarray(width_rel, np.float32)
    B, C, H, W = x.shape
    nh, nw = H // BS, W // BS
    nb = nh * nw

    # Fold the rel-embedding projections into the 1x1 conv:
    # hdot[h, pix, r] = (height_rel @ w_q_head)[r, :] . x[:, pix]
    w_rel = np.zeros((OC_REL, C), np.float32)
    for h in range(HEADS):
        wq_h = w_q[h * DH_QK:(h + 1) * DH_QK]        # (32, 256)
        w_rel[h * 64:h * 64 + 27] = height_rel @ wq_h
        w_rel[h * 64 + 28:h * 64 + 55] = width_rel @ wq_h
    # SCALE folded into the q weights: device returns SCALE*q, so the
    # host never multiplies the big logits array by SCALE.
    w_full = np.concatenate([w_q * SCALE, w_kv, w_rel], axis=0)  # (OC, 256)

    try:
        proj = _project_on_device(x, w_full)
        DEVICE_OK = True
    except Exception:
        import traceback

        traceback.print_exc()
        proj = np.einsum('bchw,oc->bohw', x, w_full).astype(np.float32)

    q = proj[:, :256]
    kv = proj[:, 256:768]
    relp = proj[:, 768:]          # (B, OC_REL, H, W)

    q = q.reshape(B * HEADS, DH_QK, nh, BS, nw, BS).transpose(0, 2, 4, 3, 5, 1)
    q = np.ascontiguousarray(q.reshape(B * HEADS, nb, BS * BS, DH_QK))
    kv = np.pad(kv, ((0, 0), (0, 0), (HALO, HALO), (HALO, HALO)))
    # single-step halo-window gather: (B, 512, nh, nw, WIN, WIN)
    ihh = (np.arange(nh)[:, None] * BS + np.arange(WIN)[None, :])  # (nh, WIN)
    iww = (np.arange(nw)[:, None] * BS + np.arange(WIN)[None, :])  # (nw, WIN)
    gi = ihh[:, None, :, None]   # (nh, 1, WIN, 1)
    gj = iww[None, :, None, :]   # (1, nw, 1, WIN)
    kv = kv[:, :, gi, gj]        # (B, 512, nh, nw, WIN, WIN)
    kv = kv.reshape(B * HEADS, DH_QK + DH_V, nb, WIN * WIN)
    k = np.ascontiguousarray(kv[:, :DH_QK].transpose(0, 2, 1, 3))   # (BH,nb,32,196)
    v = np.ascontiguousarray(kv[:, DH_QK:].transpose(0, 2, 3, 1))   # (BH,nb,196,32)

    # rel dots from the device projection: select the WIN-wide band per
    # query in pixel space first (cheap strided slices; channel offset
    # depends only on y%8 resp. x%8), then block-layout just the selection.
    BH = B * HEADS
    relp = relp.reshape(BH, 64, H, W)
    hselp = np.empty((BH, WIN, H, W), np.float32)
    wselp = np.empty((BH, WIN, H, W), np.float32)
    for r in range(BS):
        hselp[:, :, r::BS, :] = relp[:, 13 - r:27 - r, r::BS, :]
        wselp[:, :, :, r::BS] = relp[:, 41 - r:55 - r, :, r::BS]
    hsel = hselp.reshape(BH, WIN, nh, BS, nw, BS).transpose(0, 2, 4, 3, 5, 1)
    hsel = hsel.reshape(BH, nb, 64, WIN)
    wsel = wselp.reshape(BH, WIN, nh, BS, nw, BS).transpose(0, 2, 4, 3, 5, 1)
    wsel = wsel.reshape(BH, nb, 64, WIN)

    attn = np.matmul(q, k)                                  # (BH,nb,64,196)
    attn5 = attn.reshape(B * HEADS, nb, 64, WIN, WIN)
    attn5 += hsel[:, :, :, :, None]
    attn5 += wsel[:, :, :, None, :]
    np.exp(attn, out=attn)
    s = attn.sum(axis=-1, keepdims=True)
    out = np.matmul(attn, v)
    out /= s

    out = out.transpose(0, 3, 2, 1)
    out = out.reshape(-1, BS, BS, nh, nw).transpose(0, 3, 1, 4, 2)
    out = np.ascontiguousarray(out.reshape(B, HEADS * DH_V, H, W), dtype=np.float32)
    return out
